# revision 19
# baseline (speedup 1.0000x reference)
"""Conv2d 3x3 (B=32, Cin=128, H=W=56, Cout=256, pad=1, stride=1) + bias.

Strategy: data-parallel over batch across 8 NeuronCores (4 images/core).
Per core, implicit-GEMM conv: for each output tile of 8 rows x 56 cols,
accumulate 9 shifted matmuls (one per kernel tap) into PSUM:
  out[co, y, x] = sum_{ky,kx} W[ky,kx][ci, :].T @ x[ci, y+ky-1, x+kx-1]

x is staged in SBUF as overlapping row bands [128, 10, W'] (one per output
row tile, rows 8t-1..8t+8), so compute starts as soon as the first band
lands and band DMAs stream fully contiguously. Vertical padding = zeroed
pad rows in the edge bands; horizontal padding = either zeroed pad columns
(fp32r variant, whose ISA rules demand even flat patterns) or clipped
column sub-ranges with PSUM has_written first-touch-overwrite semantics
(16-bit variant).

Weights are host-pretransposed to [ci, cb, tap, co_l] so each tap's lhsT
([K=ci partitions, M=co_l]) is a contiguous SBUF slice.
"""

import numpy as np

import concourse.bass as bass
import concourse.mybir as mybir
import concourse.tile as tile
from concourse import bacc
from concourse.bass_utils import run_bass_kernel_spmd

B, C_IN, H, W = 32, 128, 56, 56
C_OUT, KSZ = 256, 3
N_CORES = 8
B_LOC = B // N_CORES  # 4 images per core
RT = 8  # output rows per tile
NT = H // RT  # 7 row tiles
CBLKS = C_OUT // 128  # 2

MM_DT = mybir.dt.float16
NP_DT = {mybir.dt.float16: np.float16, mybir.dt.bfloat16: None, mybir.dt.float32r: np.float32}


def _np_cast(a, mm_dt):
    if mm_dt == mybir.dt.float16:
        return np.ascontiguousarray(a, dtype=np.float16)
    if mm_dt == mybir.dt.bfloat16:
        import ml_dtypes

        return np.ascontiguousarray(a.astype(ml_dtypes.bfloat16))
    return np.ascontiguousarray(a, dtype=np.float32)


def build_nc(mm_dt=MM_DT):
    padded = mm_dt == mybir.dt.float32r
    BC = 58 if padded else 56  # band columns
    nc = bacc.Bacc(None, target_bir_lowering=False)
    x = nc.dram_tensor("x", [B_LOC, C_IN, H, W], mm_dt, kind="ExternalInput")
    wt = nc.dram_tensor("wt", [C_IN, CBLKS, KSZ * KSZ, 128], mm_dt, kind="ExternalInput")
    bias = nc.dram_tensor("bias", [128, CBLKS], mybir.dt.float32, kind="ExternalInput")
    out = nc.dram_tensor("out", [B_LOC, C_OUT, H, W], mybir.dt.float32, kind="ExternalOutput")

    with tile.TileContext(nc) as tc:
        with (
            tc.tile_pool(name="xin", bufs=6) as xpool,
            tc.tile_pool(name="wpool", bufs=1) as wpool,
            tc.tile_pool(name="psum", bufs=7, space="PSUM") as psum_pool,
            tc.tile_pool(name="outp", bufs=6) as opool,
        ):
            # weights + bias on the scalar DMA ring (sync ring carries x bands)
            w_sb = wpool.tile([C_IN, CBLKS, KSZ * KSZ, 128], mm_dt)
            for cb in range(CBLKS):
                nc.scalar.dma_start(w_sb[:, cb], wt[:, cb])
            bias_sb = wpool.tile([128, CBLKS], mybir.dt.float32)
            nc.scalar.dma_start(bias_sb[:], bias[:, :])

            # HAM pre-warm: dummy matmuls (on the freshly landed weights, into
            # a scratch PSUM bank nobody reads) keep the PE busy during the
            # initial DMA wait so the real stream starts at 2.4GHz and the PE
            # IRAM first-fetch stall is hidden.
            warm_ps = psum_pool.tile([128, 256], mybir.dt.float32, name="warm_ps", bufs=1)
            for _ in range(24):
                nc.tensor.matmul(
                    warm_ps[:], w_sb[:, 0, 0, :], w_sb[:, 0, 0:2, :],
                    start=True, stop=True, skip_group_check=True,
                )

            def band(b, t):
                """Stage x rows 8t-1 .. 8t+8 of image b as [128, 10, BC].
                Edge bands leave their pad row uninitialized; the matmul
                windows clip those rows instead of reading zeros."""
                xt = xpool.tile([C_IN, RT + 2, BC], mm_dt)
                if padded:
                    f32view = xt[:].bitcast(mybir.dt.float32)
                    if t == 0:
                        nc.gpsimd.memset(f32view[:, 0:1, :], 0)
                    if t == NT - 1:
                        nc.gpsimd.memset(f32view[:, RT + 1 : RT + 2, :], 0)
                    nc.gpsimd.memset(f32view[:, :, 0:1], 0)
                    nc.gpsimd.memset(f32view[:, :, 57:58], 0)
                r0 = max(0, t * RT - 1)
                r1 = min(H, t * RT + RT + 1)
                l0 = 1 if t == 0 else 0
                c0 = 1 if padded else 0
                nc.sync.dma_start(
                    xt[:, l0 : l0 + (r1 - r0), c0 : c0 + W], x[b, :, r0:r1, :]
                )
                return xt

            for b in range(B_LOC):
                for t in range(NT):
                    xt = band(b, t)
                    for cb in range(CBLKS):
                        ps = psum_pool.tile([128, RT, W], mybir.dt.float32)
                        for ky in range(KSZ):
                            # clip rows that would read the uninitialized
                            # pad row of the first/last band
                            r_off = 1 if (t == 0 and ky == 0) else 0
                            nrow = RT - r_off - (
                                1 if (t == NT - 1 and ky == 2) else 0
                            )
                            for kx in range(KSZ):
                                if padded:
                                    rhs = xt[:, ky : ky + RT, kx : kx + W]
                                    dst = ps[:, :, :]
                                else:
                                    # clip columns at image edges
                                    oc0 = 1 if kx == 0 else 0
                                    ncol = W - (1 if kx != 1 else 0)
                                    ic0 = 0 if kx == 0 else kx - 1
                                    rhs = xt[
                                        :,
                                        ky + r_off : ky + r_off + nrow,
                                        ic0 : ic0 + ncol,
                                    ]
                                    dst = ps[
                                        :,
                                        r_off : r_off + nrow,
                                        oc0 : oc0 + ncol,
                                    ]
                                nc.tensor.matmul(
                                    dst,
                                    w_sb[:, cb, ky * KSZ + kx, :],
                                    rhs,
                                    start=(ky == 0 and kx == 0),
                                    stop=(ky == 2 and kx == 2),
                                    skip_group_check=True,
                                )
                        ot = opool.tile([128, RT, W], mybir.dt.float32)
                        nc.scalar.activation(
                            ot[:],
                            ps[:],
                            mybir.ActivationFunctionType.Identity,
                            bias=bias_sb[:, cb : cb + 1],
                            scale=1.0,
                        )
                        nc.scalar.dma_start(
                            out[b, cb * 128 : (cb + 1) * 128, t * RT : (t + 1) * RT, :],
                            ot[:],
                        )
    nc.finalize()
    return nc


def prep_inputs(x, weight, bias, mm_dt=MM_DT):
    # weight (256,128,3,3) -> [ci, cb, ky*kx, co_l]
    wt = (
        weight.reshape(CBLKS, 128, C_IN, KSZ, KSZ)
        .transpose(2, 0, 3, 4, 1)
        .reshape(C_IN, CBLKS, KSZ * KSZ, 128)
    )
    wt = _np_cast(wt, mm_dt)
    bias_r = np.ascontiguousarray(bias.reshape(CBLKS, 128).T, dtype=np.float32)
    in_maps = []
    for c in range(N_CORES):
        in_maps.append(
            {
                "x": _np_cast(x[c * B_LOC : (c + 1) * B_LOC], mm_dt),
                "wt": wt,
                "bias": bias_r,
            }
        )
    return in_maps


_NC_CACHE = {}


def run(x, weight, bias, trace=False, nc=None, tmpdir=None, mm_dt=MM_DT):
    if nc is None:
        nc = _NC_CACHE.get(mm_dt)
        if nc is None:
            nc = _NC_CACHE[mm_dt] = build_nc(mm_dt)
    in_maps = prep_inputs(np.asarray(x), np.asarray(weight), np.asarray(bias), mm_dt)
    res = run_bass_kernel_spmd(
        nc, in_maps, core_ids=list(range(N_CORES)), trace=trace, tmpdir=tmpdir
    )
    out = np.concatenate([r["out"] for r in res.results], axis=0)
    return out, res


def kernel(x, weight, bias):
    out, _ = run(x, weight, bias, trace=False)
    return out


if __name__ == "__main__":
    rng = np.random.default_rng(0)
    x = rng.standard_normal((B, C_IN, H, W), dtype=np.float32)
    w = (rng.standard_normal((C_OUT, C_IN, KSZ, KSZ), dtype=np.float32) * 0.05).astype(
        np.float32
    )
    b = rng.standard_normal((C_OUT,), dtype=np.float32)
    out = kernel(x, w, b)
    print(out.shape, out.dtype)


# revision 20
# speedup vs baseline: 1.0333x; 1.0333x over previous
"""Conv2d 3x3 (B=32, Cin=128, H=W=56, Cout=256, pad=1, stride=1) + bias.

Strategy: data-parallel over batch across 8 NeuronCores (4 images/core).
Per core, implicit-GEMM conv: for each output tile of 8 rows x 56 cols,
accumulate 9 shifted matmuls (one per kernel tap) into PSUM:
  out[co, y, x] = sum_{ky,kx} W[ky,kx][ci, :].T @ x[ci, y+ky-1, x+kx-1]

x is staged in SBUF as overlapping row bands [128, 10, W'] (one per output
row tile, rows 8t-1..8t+8), so compute starts as soon as the first band
lands and band DMAs stream fully contiguously. Vertical padding = zeroed
pad rows in the edge bands; horizontal padding = either zeroed pad columns
(fp32r variant, whose ISA rules demand even flat patterns) or clipped
column sub-ranges with PSUM has_written first-touch-overwrite semantics
(16-bit variant).

Weights are host-pretransposed to [ci, cb, tap, co_l] so each tap's lhsT
([K=ci partitions, M=co_l]) is a contiguous SBUF slice.
"""

import numpy as np

import concourse.bass as bass
import concourse.mybir as mybir
import concourse.tile as tile
from concourse import bacc
from concourse.bass_utils import run_bass_kernel_spmd

B, C_IN, H, W = 32, 128, 56, 56
C_OUT, KSZ = 256, 3
N_CORES = 8
B_LOC = B // N_CORES  # 4 images per core
RT = 8  # output rows per tile
NT = H // RT  # 7 row tiles
CBLKS = C_OUT // 128  # 2

MM_DT = mybir.dt.float16
NP_DT = {mybir.dt.float16: np.float16, mybir.dt.bfloat16: None, mybir.dt.float32r: np.float32}


def _np_cast(a, mm_dt):
    if mm_dt == mybir.dt.float16:
        return np.ascontiguousarray(a, dtype=np.float16)
    if mm_dt == mybir.dt.bfloat16:
        import ml_dtypes

        return np.ascontiguousarray(a.astype(ml_dtypes.bfloat16))
    return np.ascontiguousarray(a, dtype=np.float32)


def build_nc(mm_dt=MM_DT):
    padded = mm_dt == mybir.dt.float32r
    BC = 58 if padded else 56  # band columns
    nc = bacc.Bacc(None, target_bir_lowering=False)
    x = nc.dram_tensor("x", [B_LOC, C_IN, H, W], mm_dt, kind="ExternalInput")
    wt = nc.dram_tensor("wt", [C_IN, CBLKS, KSZ * KSZ, 128], mm_dt, kind="ExternalInput")
    bias = nc.dram_tensor("bias", [128, CBLKS], mybir.dt.float32, kind="ExternalInput")
    out = nc.dram_tensor("out", [B_LOC, C_OUT, H, W], mybir.dt.float32, kind="ExternalOutput")

    with tile.TileContext(nc) as tc:
        with (
            tc.tile_pool(name="xin", bufs=6) as xpool,
            tc.tile_pool(name="wpool", bufs=1) as wpool,
            tc.tile_pool(name="psum", bufs=7, space="PSUM") as psum_pool,
            tc.tile_pool(name="outp", bufs=6) as opool,
        ):
            # weights + bias on the scalar DMA ring (sync ring carries x bands)
            w_sb = wpool.tile([C_IN, CBLKS, KSZ * KSZ, 128], mm_dt)
            for cb in range(CBLKS):
                nc.scalar.dma_start(w_sb[:, cb], wt[:, cb])
            bias_sb = wpool.tile([128, CBLKS], mybir.dt.float32)
            nc.scalar.dma_start(bias_sb[:], bias[:, :])

            # HAM pre-warm: dummy matmuls on a memset scratch tile. The memset
            # runs on gpsimd BEFORE the DMA rings open, so the PE starts
            # executing at ~7.5us with no DMA dependency — hiding both the PE
            # IRAM first-fetch stall and the HAM cold ramp under the initial
            # DMA wait. (Feeding the dummies from a DMA'd tile instead re-
            # exposes a ~3.6us sequencer stall before the first matmul.)
            warm = wpool.tile([C_IN, 256], mm_dt)
            warm_ps = psum_pool.tile([128, 256], mybir.dt.float32, name="warm_ps", bufs=1)
            warm_view_dt = (
                mybir.dt.float32 if mm_dt == mybir.dt.float32r else mybir.dt.uint16
            )
            nc.gpsimd.memset(warm[:].bitcast(warm_view_dt), 0)
            for _ in range(24):
                nc.tensor.matmul(
                    warm_ps[:], warm[:, :128], warm[:, :256],
                    start=True, stop=True, skip_group_check=True,
                )

            def band(b, t):
                """Stage x rows 8t-1 .. 8t+8 of image b as [128, 10, BC].
                Edge bands leave their pad row uninitialized; the matmul
                windows clip those rows instead of reading zeros."""
                xt = xpool.tile([C_IN, RT + 2, BC], mm_dt)
                if padded:
                    f32view = xt[:].bitcast(mybir.dt.float32)
                    if t == 0:
                        nc.gpsimd.memset(f32view[:, 0:1, :], 0)
                    if t == NT - 1:
                        nc.gpsimd.memset(f32view[:, RT + 1 : RT + 2, :], 0)
                    nc.gpsimd.memset(f32view[:, :, 0:1], 0)
                    nc.gpsimd.memset(f32view[:, :, 57:58], 0)
                r0 = max(0, t * RT - 1)
                r1 = min(H, t * RT + RT + 1)
                l0 = 1 if t == 0 else 0
                c0 = 1 if padded else 0
                nc.sync.dma_start(
                    xt[:, l0 : l0 + (r1 - r0), c0 : c0 + W], x[b, :, r0:r1, :]
                )
                return xt

            for b in range(B_LOC):
                for t in range(NT):
                    xt = band(b, t)
                    for cb in range(CBLKS):
                        ps = psum_pool.tile([128, RT, W], mybir.dt.float32)
                        for ky in range(KSZ):
                            # clip rows that would read the uninitialized
                            # pad row of the first/last band
                            r_off = 1 if (t == 0 and ky == 0) else 0
                            nrow = RT - r_off - (
                                1 if (t == NT - 1 and ky == 2) else 0
                            )
                            for kx in range(KSZ):
                                if padded:
                                    rhs = xt[:, ky : ky + RT, kx : kx + W]
                                    dst = ps[:, :, :]
                                else:
                                    # clip columns at image edges
                                    oc0 = 1 if kx == 0 else 0
                                    ncol = W - (1 if kx != 1 else 0)
                                    ic0 = 0 if kx == 0 else kx - 1
                                    rhs = xt[
                                        :,
                                        ky + r_off : ky + r_off + nrow,
                                        ic0 : ic0 + ncol,
                                    ]
                                    dst = ps[
                                        :,
                                        r_off : r_off + nrow,
                                        oc0 : oc0 + ncol,
                                    ]
                                nc.tensor.matmul(
                                    dst,
                                    w_sb[:, cb, ky * KSZ + kx, :],
                                    rhs,
                                    start=(ky == 0 and kx == 0),
                                    stop=(ky == 2 and kx == 2),
                                    skip_group_check=True,
                                )
                        ot = opool.tile([128, RT, W], mybir.dt.float32)
                        nc.scalar.activation(
                            ot[:],
                            ps[:],
                            mybir.ActivationFunctionType.Identity,
                            bias=bias_sb[:, cb : cb + 1],
                            scale=1.0,
                        )
                        nc.scalar.dma_start(
                            out[b, cb * 128 : (cb + 1) * 128, t * RT : (t + 1) * RT, :],
                            ot[:],
                        )
    nc.finalize()
    return nc


def prep_inputs(x, weight, bias, mm_dt=MM_DT):
    # weight (256,128,3,3) -> [ci, cb, ky*kx, co_l]
    wt = (
        weight.reshape(CBLKS, 128, C_IN, KSZ, KSZ)
        .transpose(2, 0, 3, 4, 1)
        .reshape(C_IN, CBLKS, KSZ * KSZ, 128)
    )
    wt = _np_cast(wt, mm_dt)
    bias_r = np.ascontiguousarray(bias.reshape(CBLKS, 128).T, dtype=np.float32)
    in_maps = []
    for c in range(N_CORES):
        in_maps.append(
            {
                "x": _np_cast(x[c * B_LOC : (c + 1) * B_LOC], mm_dt),
                "wt": wt,
                "bias": bias_r,
            }
        )
    return in_maps


_NC_CACHE = {}


def run(x, weight, bias, trace=False, nc=None, tmpdir=None, mm_dt=MM_DT):
    if nc is None:
        nc = _NC_CACHE.get(mm_dt)
        if nc is None:
            nc = _NC_CACHE[mm_dt] = build_nc(mm_dt)
    in_maps = prep_inputs(np.asarray(x), np.asarray(weight), np.asarray(bias), mm_dt)
    res = run_bass_kernel_spmd(
        nc, in_maps, core_ids=list(range(N_CORES)), trace=trace, tmpdir=tmpdir
    )
    out = np.concatenate([r["out"] for r in res.results], axis=0)
    return out, res


def kernel(x, weight, bias):
    out, _ = run(x, weight, bias, trace=False)
    return out


if __name__ == "__main__":
    rng = np.random.default_rng(0)
    x = rng.standard_normal((B, C_IN, H, W), dtype=np.float32)
    w = (rng.standard_normal((C_OUT, C_IN, KSZ, KSZ), dtype=np.float32) * 0.05).astype(
        np.float32
    )
    b = rng.standard_normal((C_OUT,), dtype=np.float32)
    out = kernel(x, w, b)
    print(out.shape, out.dtype)


# revision 21
# speedup vs baseline: 1.0346x; 1.0012x over previous
"""Conv2d 3x3 (B=32, Cin=128, H=W=56, Cout=256, pad=1, stride=1) + bias.

Strategy: data-parallel over batch across 8 NeuronCores (4 images/core).
Per core, implicit-GEMM conv: for each output tile of 8 rows x 56 cols,
accumulate 9 shifted matmuls (one per kernel tap) into PSUM:
  out[co, y, x] = sum_{ky,kx} W[ky,kx][ci, :].T @ x[ci, y+ky-1, x+kx-1]

x is staged in SBUF as overlapping row bands [128, 10, W'] (one per output
row tile, rows 8t-1..8t+8), so compute starts as soon as the first band
lands and band DMAs stream fully contiguously. Vertical padding = zeroed
pad rows in the edge bands; horizontal padding = either zeroed pad columns
(fp32r variant, whose ISA rules demand even flat patterns) or clipped
column sub-ranges with PSUM has_written first-touch-overwrite semantics
(16-bit variant).

Weights are host-pretransposed to [ci, cb, tap, co_l] so each tap's lhsT
([K=ci partitions, M=co_l]) is a contiguous SBUF slice.
"""

import numpy as np

import concourse.bass as bass
import concourse.mybir as mybir
import concourse.tile as tile
from concourse import bacc
from concourse.bass_utils import run_bass_kernel_spmd

B, C_IN, H, W = 32, 128, 56, 56
C_OUT, KSZ = 256, 3
N_CORES = 8
B_LOC = B // N_CORES  # 4 images per core
RT = 8  # output rows per tile
NT = H // RT  # 7 row tiles
CBLKS = C_OUT // 128  # 2

MM_DT = mybir.dt.float16
NP_DT = {mybir.dt.float16: np.float16, mybir.dt.bfloat16: None, mybir.dt.float32r: np.float32}


def _np_cast(a, mm_dt):
    if mm_dt == mybir.dt.float16:
        return np.ascontiguousarray(a, dtype=np.float16)
    if mm_dt == mybir.dt.bfloat16:
        import ml_dtypes

        return np.ascontiguousarray(a.astype(ml_dtypes.bfloat16))
    return np.ascontiguousarray(a, dtype=np.float32)


def build_nc(mm_dt=MM_DT):
    padded = mm_dt == mybir.dt.float32r
    BC = 58 if padded else 56  # band columns
    nc = bacc.Bacc(None, target_bir_lowering=False)
    x = nc.dram_tensor("x", [B_LOC, C_IN, H, W], mm_dt, kind="ExternalInput")
    wt = nc.dram_tensor("wt", [C_IN, CBLKS, KSZ * KSZ, 128], mm_dt, kind="ExternalInput")
    bias = nc.dram_tensor("bias", [128, CBLKS], mybir.dt.float32, kind="ExternalInput")
    out = nc.dram_tensor("out", [B_LOC, C_OUT, H, W], mybir.dt.float32, kind="ExternalOutput")

    with tile.TileContext(nc) as tc:
        with (
            tc.tile_pool(name="xin", bufs=6) as xpool,
            tc.tile_pool(name="wpool", bufs=1) as wpool,
            tc.tile_pool(name="psum", bufs=7, space="PSUM") as psum_pool,
            tc.tile_pool(name="outp", bufs=6) as opool,
        ):
            # weights + bias on the scalar DMA ring (sync ring carries x bands)
            w_sb = wpool.tile([C_IN, CBLKS, KSZ * KSZ, 128], mm_dt)
            for cb in range(CBLKS):
                nc.scalar.dma_start(w_sb[:, cb], wt[:, cb])
            bias_sb = wpool.tile([128, CBLKS], mybir.dt.float32)
            nc.scalar.dma_start(bias_sb[:], bias[:, :])

            # HAM pre-warm: dummy matmuls on a memset scratch tile. The memset
            # runs on gpsimd BEFORE the DMA rings open, so the PE starts
            # executing at ~7.5us with no DMA dependency — hiding both the PE
            # IRAM first-fetch stall and the HAM cold ramp under the initial
            # DMA wait. (Feeding the dummies from a DMA'd tile instead re-
            # exposes a ~3.6us sequencer stall before the first matmul.)
            warm = wpool.tile([C_IN, 256], mm_dt)
            warm_ps = psum_pool.tile([128, 256], mybir.dt.float32, name="warm_ps", bufs=1)
            warm_view_dt = (
                mybir.dt.float32 if mm_dt == mybir.dt.float32r else mybir.dt.uint16
            )
            nc.gpsimd.memset(warm[:].bitcast(warm_view_dt), 0)
            for _ in range(20):
                nc.tensor.matmul(
                    warm_ps[:], warm[:, :128], warm[:, :256],
                    start=True, stop=True, skip_group_check=True,
                )

            def band(b, t):
                """Stage x rows 8t-1 .. 8t+8 of image b as [128, 10, BC].
                Edge bands leave their pad row uninitialized; the matmul
                windows clip those rows instead of reading zeros."""
                xt = xpool.tile([C_IN, RT + 2, BC], mm_dt)
                if padded:
                    f32view = xt[:].bitcast(mybir.dt.float32)
                    if t == 0:
                        nc.gpsimd.memset(f32view[:, 0:1, :], 0)
                    if t == NT - 1:
                        nc.gpsimd.memset(f32view[:, RT + 1 : RT + 2, :], 0)
                    nc.gpsimd.memset(f32view[:, :, 0:1], 0)
                    nc.gpsimd.memset(f32view[:, :, 57:58], 0)
                r0 = max(0, t * RT - 1)
                r1 = min(H, t * RT + RT + 1)
                l0 = 1 if t == 0 else 0
                c0 = 1 if padded else 0
                nc.sync.dma_start(
                    xt[:, l0 : l0 + (r1 - r0), c0 : c0 + W], x[b, :, r0:r1, :]
                )
                return xt

            for b in range(B_LOC):
                for t in range(NT):
                    xt = band(b, t)
                    for cb in range(CBLKS):
                        ps = psum_pool.tile([128, RT, W], mybir.dt.float32)
                        for ky in range(KSZ):
                            # clip rows that would read the uninitialized
                            # pad row of the first/last band
                            r_off = 1 if (t == 0 and ky == 0) else 0
                            nrow = RT - r_off - (
                                1 if (t == NT - 1 and ky == 2) else 0
                            )
                            for kx in range(KSZ):
                                if padded:
                                    rhs = xt[:, ky : ky + RT, kx : kx + W]
                                    dst = ps[:, :, :]
                                else:
                                    # clip columns at image edges
                                    oc0 = 1 if kx == 0 else 0
                                    ncol = W - (1 if kx != 1 else 0)
                                    ic0 = 0 if kx == 0 else kx - 1
                                    rhs = xt[
                                        :,
                                        ky + r_off : ky + r_off + nrow,
                                        ic0 : ic0 + ncol,
                                    ]
                                    dst = ps[
                                        :,
                                        r_off : r_off + nrow,
                                        oc0 : oc0 + ncol,
                                    ]
                                nc.tensor.matmul(
                                    dst,
                                    w_sb[:, cb, ky * KSZ + kx, :],
                                    rhs,
                                    start=(ky == 0 and kx == 0),
                                    stop=(ky == 2 and kx == 2),
                                    skip_group_check=True,
                                )
                        ot = opool.tile([128, RT, W], mybir.dt.float32)
                        nc.scalar.activation(
                            ot[:],
                            ps[:],
                            mybir.ActivationFunctionType.Identity,
                            bias=bias_sb[:, cb : cb + 1],
                            scale=1.0,
                        )
                        nc.scalar.dma_start(
                            out[b, cb * 128 : (cb + 1) * 128, t * RT : (t + 1) * RT, :],
                            ot[:],
                        )
    nc.finalize()
    return nc


def prep_inputs(x, weight, bias, mm_dt=MM_DT):
    # weight (256,128,3,3) -> [ci, cb, ky*kx, co_l]
    wt = (
        weight.reshape(CBLKS, 128, C_IN, KSZ, KSZ)
        .transpose(2, 0, 3, 4, 1)
        .reshape(C_IN, CBLKS, KSZ * KSZ, 128)
    )
    wt = _np_cast(wt, mm_dt)
    bias_r = np.ascontiguousarray(bias.reshape(CBLKS, 128).T, dtype=np.float32)
    in_maps = []
    for c in range(N_CORES):
        in_maps.append(
            {
                "x": _np_cast(x[c * B_LOC : (c + 1) * B_LOC], mm_dt),
                "wt": wt,
                "bias": bias_r,
            }
        )
    return in_maps


_NC_CACHE = {}


def run(x, weight, bias, trace=False, nc=None, tmpdir=None, mm_dt=MM_DT):
    if nc is None:
        nc = _NC_CACHE.get(mm_dt)
        if nc is None:
            nc = _NC_CACHE[mm_dt] = build_nc(mm_dt)
    in_maps = prep_inputs(np.asarray(x), np.asarray(weight), np.asarray(bias), mm_dt)
    res = run_bass_kernel_spmd(
        nc, in_maps, core_ids=list(range(N_CORES)), trace=trace, tmpdir=tmpdir
    )
    out = np.concatenate([r["out"] for r in res.results], axis=0)
    return out, res


def kernel(x, weight, bias):
    out, _ = run(x, weight, bias, trace=False)
    return out


if __name__ == "__main__":
    rng = np.random.default_rng(0)
    x = rng.standard_normal((B, C_IN, H, W), dtype=np.float32)
    w = (rng.standard_normal((C_OUT, C_IN, KSZ, KSZ), dtype=np.float32) * 0.05).astype(
        np.float32
    )
    b = rng.standard_normal((C_OUT,), dtype=np.float32)
    out = kernel(x, w, b)
    print(out.shape, out.dtype)
